# revision 11
# baseline (speedup 1.0000x reference)
"""Trainium2 Bass kernel for the fused candidate-attention module.

Computation (reference, fp32):
    ds[n,m,l]  = self_delta[n,m,l,0] + self_delta[n,m,l,1]
    out[n,l]   = sum_m value_w[m] * ds[n,m,l] * (emb[1+l,:] . self_attn[n,m,:])

Sharding: data-parallel over N (batch) across the 8 cores, B = 4 batches per
core, full L = 8192 candidates each.  Host pre-splits self_delta into its two
K-planes and casts them to fp16 (numerically identical to the fp16 cast the
device pipeline uses anyway), so every per-core DRAM slice is contiguous and
the per-core HBM traffic is ~13 MB of delta + 2 MB of emb.

Per-core device pipeline, per batch n:

    dds  [m=100, L]      <- HWDGE DMA of k-plane 0, then SWDGE accum-DMA
                            (accum_op=add) of k-plane 1: the K-sum happens in
                            the DMA datapath, not on an engine
    w    [m=100, d=128]  = value_w[m] * self_attn[n]      (DVE, fp16 out)
    g    [d=128, 512]    = w^T @ dds chunk                (PE, one matmul)
    prod [d=128, 512]    = g * embT chunk                 (DVE; either directly
                            from PSUM at 1x, or via an ACT fp16 eviction and a
                            2x DVE multiply -- K_MULDVE picks the mix)
    row  [32j..,  512]   = ones32^T @ prod chunk          (PE col-group j=h%4,
                            4 chunks packed per PSUM bank)
    out_sb               <- ACT evicts each packed bank   (4 chunks per copy)

Output leaves as 4 single-partition DMAs ([1, B*4*512] each); the host
un-permutes the (colgroup, bank) packing back to [N, L].

Env knobs: K_KSUM=dma|mm (accum-DMA vs two accumulating matmuls),
K_MULDVE=<0..16> (chunks per 16 on the direct-DVE path), K_DDBUFS, K_GBUFS,
KERNEL_STAGE=dma|mm|mul|full (pipeline prefix, for bench), KERNEL_LOOP=<R>
(device-side repeat, for bench).
"""

import os
from contextlib import ExitStack

import numpy as np

import concourse.bacc as bacc
import concourse.bass as bass
import concourse.mybir as mybir
from concourse.bass_utils import run_bass_kernel_spmd
from concourse.tile import TileContext

N, M, L, K, D = 32, 100, 8192, 2, 128
NCORES = 8
B = N // NCORES  # batches per core
MMF = 512  # matmul moving free dim (one PSUM bank of fp32)
NCHUNK = L // MMF  # 16
NBANKQ = 4  # row-reduce chunks packed per PSUM bank (4 col groups)
NQ = NCHUNK // NBANKQ  # row banks per batch

F32 = mybir.dt.float32
F16 = mybir.dt.float16

KSUM = os.environ.get("K_KSUM", "dma")
DMASPLIT = int(os.environ.get("K_DMASPLIT", "1"))
RING = os.environ.get("K_RING", "2")  # "2"=sync+gpsimd, "1"=sync, "g"=gpsimd
MULDVE = int(os.environ.get("K_MULDVE", "8"))
DDBUFS = int(os.environ.get("K_DDBUFS", "3"))
GBUFS = int(os.environ.get("K_GBUFS", "4"))
LOOP_R = int(os.environ.get("KERNEL_LOOP", "1"))
STAGE = os.environ.get("KERNEL_STAGE", "full")

# spread the direct-DVE chunks evenly through the 16 so ACT/DVE interleave
_DIRECT = set()
if MULDVE > 0:
    _DIRECT = {int(round(i * NCHUNK / MULDVE)) % NCHUNK for i in range(MULDVE)}


def _build_nc() -> bass.Bass:
    nc = bacc.Bacc()

    dd0 = nc.declare_dram_parameter("dd0", [B, M, L], F16, isOutput=False)
    dd1 = nc.declare_dram_parameter("dd1", [B, M, L], F16, isOutput=False)
    attnT = nc.declare_dram_parameter("attnT", [M, B * D], F32, isOutput=False)
    embT = nc.declare_dram_parameter("embT", [D, L], F16, isOutput=False)
    vw = nc.declare_dram_parameter("vw", [M, 1], F32, isOutput=False)
    # outp[j, (b*NQ + q)*MMF + f] = out[b, (q*NBANKQ + j)*MMF + f]
    outp = nc.declare_dram_parameter("outp", [NBANKQ, B * NQ * MMF], F32, isOutput=True)

    with TileContext(nc) as tc, ExitStack() as ctx:
        const = ctx.enter_context(tc.tile_pool(name="const", bufs=1))

        vw_sb = const.tile([M, 1], F32)
        nc.scalar.dma_start(out=vw_sb[:], in_=vw[:])
        attnT_sb = const.tile([M, B * D], F32)
        nc.scalar.dma_start(out=attnT_sb[:], in_=attnT[:])
        embT_sb = const.tile([D, L], F16)
        nc.scalar.dma_start(out=embT_sb[:], in_=embT[:])
        ones32 = const.tile([D, 32], F16)
        nc.vector.memset(ones32[:], 1.0)
        out_sb = const.tile([D, B * NQ * MMF], F32)

        dds_pool = ctx.enter_context(tc.tile_pool(name="dds", bufs=DDBUFS))
        dd1_pool = (
            ctx.enter_context(tc.tile_pool(name="dd1", bufs=DDBUFS))
            if KSUM == "mm"
            else None
        )
        w_pool = ctx.enter_context(tc.tile_pool(name="w", bufs=2))
        g_psum = ctx.enter_context(tc.tile_pool(name="g", bufs=GBUFS, space="PSUM"))
        gs_pool = ctx.enter_context(tc.tile_pool(name="gs", bufs=3))
        prod_pool = ctx.enter_context(tc.tile_pool(name="prod", bufs=3))
        row_psum = ctx.enter_context(tc.tile_pool(name="row", bufs=2, space="PSUM"))

        def emit_rows(b, prod):
            for q in range(NQ):
                row = row_psum.tile([D, MMF], F32, tag="row")
                for j in range(NBANKQ):
                    h = q * NBANKQ + j
                    lsl = slice(h * MMF, (h + 1) * MMF)
                    nc.tensor.matmul(
                        row[32 * j : 32 * j + 32, :],
                        lhsT=ones32[:],
                        rhs=prod[:, lsl],
                        start=True,
                        stop=True,
                        tile_position=(0, 32 * j),
                    )
                nc.scalar.copy(
                    out_sb[:, (b * NQ + q) * MMF : (b * NQ + q + 1) * MMF], row[:]
                )

        loop_ctx = tc.For_i(0, LOOP_R, 1) if LOOP_R > 1 else None
        if loop_ctx is not None:
            ctx.enter_context(loop_ctx)
        pending = []  # (b, prod) whose row phase is deferred one batch
        for b in range(B):
            dds = dds_pool.tile([M, L], F16)
            if KSUM == "dma":
                nc.sync.dma_start(out=dds[:], in_=dd0[b])
                # CCE (the inline DMA adder) tops out at 2048 elements per
                # descriptor; cap the descriptor run length accordingly
                nc.gpsimd.dma_start(
                    out=dds[:],
                    in_=dd1[b],
                    accum_op=mybir.AluOpType.add,
                    max_dma_last_dim=2048,
                )
                rhs_list = [dds]
            else:
                # keep delta DMAs off the ACT ring (busy with evictions);
                # split/ring policy is an empirical knob
                dd1t = dd1_pool.tile([M, L], F16)
                LS = L // DMASPLIT
                for i, (t, src) in enumerate(((dds, dd0), (dd1t, dd1))):
                    for s in range(DMASPLIT):
                        csl = slice(s * LS, (s + 1) * LS)
                        idx = b * 2 * DMASPLIT + i * DMASPLIT + s
                        if RING == "1":
                            eng = nc.sync
                        elif RING == "g":
                            eng = nc.gpsimd
                        else:
                            eng = nc.sync if idx % 2 == 0 else nc.gpsimd
                        eng.dma_start(out=t[:, csl], in_=src[b][:, csl])
                rhs_list = [dds, dd1t]
            if STAGE == "dma":
                continue

            w_t = w_pool.tile([M, D], F16)
            nc.vector.tensor_scalar(
                out=w_t[:],
                in0=attnT_sb[:, b * D : (b + 1) * D],
                scalar1=vw_sb[:, 0:1],
                scalar2=None,
                op0=mybir.AluOpType.mult,
            )

            prod = prod_pool.tile([D, L], F16)
            for h in range(NCHUNK):
                lsl = slice(h * MMF, (h + 1) * MMF)
                g = g_psum.tile([D, MMF], F32)
                for ki, r in enumerate(rhs_list):
                    nc.tensor.matmul(
                        g[:],
                        lhsT=w_t[:],
                        rhs=r[:, lsl],
                        start=(ki == 0),
                        stop=(ki == len(rhs_list) - 1),
                    )
                if STAGE == "mm":
                    continue
                if h in _DIRECT:
                    nc.vector.tensor_mul(prod[:, lsl], g[:], embT_sb[:, lsl])
                else:
                    gs = gs_pool.tile([D, MMF], F16)
                    nc.scalar.copy(gs[:], g[:])
                    nc.vector.tensor_mul(prod[:, lsl], gs[:], embT_sb[:, lsl])
            if STAGE in ("mm", "mul"):
                continue

            # defer this batch's row phase until after the next batch's
            # mul phase: its deps then precede it in every engine FIFO, so
            # neither ACT nor PE head-of-line-blocks on the pipeline tail
            pending.append((b, prod))
            if len(pending) > 1:
                emit_rows(*pending.pop(0))
        for bp in pending:
            emit_rows(*bp)

        if STAGE == "full":
            # on the ACT ring: their deps (ACT row evictions) precede them in
            # FIFO order, so they never stall the ring; on sync they would
            # block the next iteration's delta prefetch
            for j in range(NBANKQ):
                nc.scalar.dma_start(
                    out=outp[j], in_=out_sb[32 * j : 32 * j + 1, :]
                )

    nc.compile()
    return nc


_NC_CACHE: dict[str, bass.Bass] = {}


def _get_nc() -> bass.Bass:
    key = f"{KSUM}:{MULDVE}:{DDBUFS}:{GBUFS}:{LOOP_R}:{STAGE}"
    if key not in _NC_CACHE:
        _NC_CACHE[key] = _build_nc()
    return _NC_CACHE[key]


def make_in_maps(self_attn, self_delta, emb_table, value_w):
    self_attn = np.asarray(self_attn, dtype=np.float32)
    self_delta = np.asarray(self_delta, dtype=np.float32)
    emb_table = np.asarray(emb_table, dtype=np.float32)
    value_w = np.ascontiguousarray(
        np.asarray(value_w, dtype=np.float32).reshape(M, 1)
    )
    assert self_attn.shape == (N, M, D), self_attn.shape
    assert self_delta.shape == (N, M, L, K), self_delta.shape
    assert emb_table.shape == (L + 1, D), emb_table.shape

    dd0 = self_delta[:, :, :, 0].astype(np.float16)
    dd1 = self_delta[:, :, :, 1].astype(np.float16)
    embT16 = np.ascontiguousarray(emb_table[1:].T.astype(np.float16))

    in_maps = []
    for c in range(NCORES):
        nsl = slice(c * B, (c + 1) * B)
        in_maps.append(
            {
                "dd0": dd0[nsl],
                "dd1": dd1[nsl],
                "attnT": np.ascontiguousarray(
                    self_attn[nsl].transpose(1, 0, 2)
                ).reshape(M, B * D),
                "embT": embT16,
                "vw": value_w,
            }
        )
    return in_maps


def decode_out(res_outp: np.ndarray) -> np.ndarray:
    """[NBANKQ, B*NQ*MMF] staged layout -> [B, L] per core."""
    return (
        res_outp.reshape(NBANKQ, B, NQ, MMF)
        .transpose(1, 2, 0, 3)
        .reshape(B, L)
        .astype(np.float32)
    )


def kernel(self_attn, self_delta, emb_table, value_w, traj_len=None, loc_max=None):
    """Full inputs in, full output out.  traj_len is unused by the reference."""
    if loc_max is not None:
        assert int(loc_max) == L, loc_max
    in_maps = make_in_maps(self_attn, self_delta, emb_table, value_w)

    nc = _get_nc()
    try:
        res = run_bass_kernel_spmd(nc, in_maps, list(range(NCORES)))
    except Exception:
        # one retry for transient NRT execution failures
        res = run_bass_kernel_spmd(nc, in_maps, list(range(NCORES)))
    out = np.concatenate(
        [decode_out(res.results[c]["outp"]) for c in range(NCORES)], axis=0
    )
    return out
